# revision 17
# baseline (speedup 1.0000x reference)
"""MOLELinear (mixture-of-linear-experts) Trainium2 kernel.

Math (per group g): out_g = x_g @ (sum_e c[g,e] W_e + W_sh).T + (sum_e c[g,e] b_e + b_sh)

Sharding: data-parallel over the 32 groups -> 4 groups (8192 tokens) per core,
expert weights replicated. Host does layout-only prep (transposes / stacking, no
arithmetic); all FLOPs (weight mixing, bias mixing, GEMM, bias add) run on device.

Device plan per core:
  - DMA in: xT [512, 8192] (x shard transposed), WT [9, 512, 512] (transposed
    experts + shared), coefficient broadcast / bias tensors.
  - Mix weights on DVE: WmixT_g = sum_e c[g,e] WT_e + WT_sh via fused
    scalar_tensor_tensor FMAs (8 ops of [128, 2048] per group).
  - Mix biases on PE: tiny K=9 matmul per group.
  - Main GEMM on PE in float32r (1 cyc/row): psum[t128, o512] accumulates 4
    k-tiles plus a K=1 ones-row matmul that adds the mixed bias.
  - Drain PSUM->SBUF on ScalarE, DMA out.
"""
import ml_dtypes
import numpy as np

import concourse.bacc as bacc
import concourse.mybir as mybir
from concourse.alu_op_type import AluOpType
from concourse.tile import TileContext
from concourse.bass_utils import run_bass_kernel_spmd

N_CORES = 8
IN_F = 512
OUT_F = 512
N_EXPERTS = 8
N_GROUPS = 32
TOK_PER_GROUP = 2048
G_PER_CORE = N_GROUPS // N_CORES          # 4
TOK_PER_CORE = G_PER_CORE * TOK_PER_GROUP  # 8192
KT = IN_F // 128                           # 4 k-tiles
F32 = mybir.dt.float32
F32R = mybir.dt.float32r
BF16 = mybir.dt.bfloat16

_CACHE = {}


def _build():
    nc = bacc.Bacc(trn_type="TRN2")
    xT = nc.dram_tensor("xT", (IN_F, TOK_PER_CORE), F32, kind="ExternalInput")
    wt = nc.dram_tensor("wt", (N_EXPERTS + 1, IN_F, OUT_F), F32R, kind="ExternalInput")
    cb = nc.dram_tensor("cb", (128, G_PER_CORE * N_EXPERTS), F32, kind="ExternalInput")
    cx = nc.dram_tensor("cx", (N_EXPERTS + 1, G_PER_CORE), F32R, kind="ExternalInput")
    ball = nc.dram_tensor("ball", (N_EXPERTS + 1, OUT_F), F32R, kind="ExternalInput")
    ones = nc.dram_tensor("ones", (1, 128), BF16, kind="ExternalInput")
    out = nc.dram_tensor("out", (TOK_PER_CORE, OUT_F), F32, kind="ExternalOutput")

    with TileContext(nc) as tc:
        with (
            tc.tile_pool(name="wp", bufs=1) as wp,
            tc.tile_pool(name="mixp", bufs=1) as mixp,
            tc.tile_pool(name="smallp", bufs=1) as smallp,
            tc.tile_pool(name="xp", bufs=3) as xp,
            tc.tile_pool(name="op", bufs=3) as op,
            tc.tile_pool(name="psp", bufs=6, space="PSUM") as psp,
            tc.tile_pool(name="psb", bufs=2, space="PSUM") as psb,
        ):
            # ---- load experts per k-tile (kt-major so mixing can start early) ----
            wsb = {}
            for kt in range(KT):
                for e in range(N_EXPERTS + 1):
                    t = wp.tile([128, OUT_F], F32R, tag=f"w{e}_{kt}")
                    nc.sync.dma_start(t[:], wt[e][kt * 128 : (kt + 1) * 128, :])
                    wsb[(e, kt)] = t

            cbt = smallp.tile([128, G_PER_CORE * N_EXPERTS], F32, tag="cb")
            nc.sync.dma_start(cbt[:], cb[:])
            cxt = smallp.tile([N_EXPERTS + 1, G_PER_CORE], F32R, tag="cx")
            nc.sync.dma_start(cxt[:], cx[:])
            ballt = smallp.tile([N_EXPERTS + 1, OUT_F], F32R, tag="ball")
            nc.sync.dma_start(ballt[:], ball[:])
            onest = smallp.tile([1, 128], BF16, tag="ones")
            nc.sync.dma_start(onest[:], ones[:])

            # ---- mixed biases: mb_g = cx[:, g].T @ ball  (K=9, M=1, N=512) ----
            mbt = []
            for g in range(G_PER_CORE):
                pbg = psb.tile([1, OUT_F], F32, tag="pb")
                nc.tensor.matmul(pbg[:], cxt[:, g : g + 1], ballt[:], start=True, stop=True)
                mb = smallp.tile([1, OUT_F], BF16, tag=f"mb{g}")
                nc.vector.tensor_copy(mb[:], pbg[:])
                mbt.append(mb)

            # ---- mix weights on DVE: wmix_g = sum_e c[g,e]*WT_e + WT_sh ----
            # ---- mix per (group, k-tile); final FMA writes bf16 ----
            wmix = {}
            for g in range(G_PER_CORE):
                for kt in range(KT):
                    acc = mixp.tile([128, OUT_F], F32, tag="wma", bufs=2)
                    wm = mixp.tile([128, OUT_F], BF16, tag=f"wm{g}_{kt}")
                    nc.vector.scalar_tensor_tensor(
                        acc[:], wsb[(0, kt)][:],
                        cbt[:, g * N_EXPERTS : g * N_EXPERTS + 1],
                        wsb[(N_EXPERTS, kt)][:], AluOpType.mult, AluOpType.add,
                    )
                    for e in range(1, N_EXPERTS):
                        nc.vector.scalar_tensor_tensor(
                            acc[:] if e < N_EXPERTS - 1 else wm[:],
                            wsb[(e, kt)][:],
                            cbt[:, g * N_EXPERTS + e : g * N_EXPERTS + e + 1],
                            acc[:], AluOpType.mult, AluOpType.add,
                        )
                    wmix[(g, kt)] = wm

            # ---- main GEMM ----
            n_chunks = TOK_PER_CORE // 512  # 16 chunks of 512 tokens
            for ch in range(n_chunks):
                g = ch // (TOK_PER_GROUP // 512)
                t0 = ch * 512
                xs = xp.tile([128, KT * 512], F32, tag="x")
                nc.sync.dma_start(
                    xs[:].rearrange("p (kt t) -> p kt t", kt=KT),
                    xT[:, t0 : t0 + 512].rearrange("(kt p) t -> p kt t", p=128),
                )
                xb = xp.tile([128, KT * 512], BF16, tag="xb")
                nc.scalar.copy(xb[:], xs[:])
                oc = op.tile([128, 4 * OUT_F], F32, tag="o")
                for ts in range(4):
                    ps = psp.tile([128, OUT_F], F32, tag="ps")
                    for kt in range(KT):
                        nc.tensor.matmul(
                            ps[:],
                            xb[:, kt * 512 + ts * 128 : kt * 512 + ts * 128 + 128],
                            wmix[(g, kt)][:],
                            start=(kt == 0),
                            stop=False,
                        )
                    nc.tensor.matmul(ps[:], onest[:], mbt[g][:], start=False, stop=True)
                    nc.scalar.copy(oc[:, ts * OUT_F : (ts + 1) * OUT_F], ps[:])
                nc.sync.dma_start(
                    out[t0 : t0 + 512, :].rearrange("(ts p) o -> p ts o", p=128),
                    oc[:].rearrange("p (ts o) -> p ts o", ts=4),
                )
    nc.finalize()
    return nc


def kernel(x, coefficients, weight_experts, bias_experts, weight_shared, bias_shared, sizes):
    x = np.asarray(x)
    coefficients = np.asarray(coefficients)
    weight_experts = np.asarray(weight_experts)
    bias_experts = np.asarray(bias_experts)
    weight_shared = np.asarray(weight_shared)
    bias_shared = np.asarray(bias_shared)

    if "nc" not in _CACHE:
        _CACHE["nc"] = _build()
    nc = _CACHE["nc"]

    # ---- host-side layout prep (no arithmetic) ----
    wt_np = np.empty((N_EXPERTS + 1, IN_F, OUT_F), np.float32)
    for e in range(N_EXPERTS):
        wt_np[e] = weight_experts[e].T
    wt_np[N_EXPERTS] = weight_shared.T
    ball_np = np.empty((N_EXPERTS + 1, OUT_F), np.float32)
    ball_np[:N_EXPERTS] = bias_experts
    ball_np[N_EXPERTS] = bias_shared
    ones_np = np.ones((1, 128), ml_dtypes.bfloat16)

    in_maps = []
    for c in range(N_CORES):
        gs = slice(c * G_PER_CORE, (c + 1) * G_PER_CORE)
        cg = coefficients[gs]  # [4, 8]
        cb_np = np.broadcast_to(
            cg.reshape(1, -1), (128, G_PER_CORE * N_EXPERTS)
        ).copy()
        cx_np = np.empty((N_EXPERTS + 1, G_PER_CORE), np.float32)
        cx_np[:N_EXPERTS] = cg.T
        cx_np[N_EXPERTS] = 1.0
        xT_np = np.ascontiguousarray(
            x[c * TOK_PER_CORE : (c + 1) * TOK_PER_CORE].T
        )
        in_maps.append(
            {
                "xT": xT_np,
                "wt": wt_np,
                "cb": cb_np,
                "cx": cx_np,
                "ball": ball_np,
                "ones": ones_np,
            }
        )

    res = run_bass_kernel_spmd(nc, in_maps, core_ids=list(range(N_CORES)))
    return np.concatenate([res.results[c]["out"] for c in range(N_CORES)], axis=0)


# revision 19
# speedup vs baseline: 1.1235x; 1.1235x over previous
"""MOLELinear (mixture-of-linear-experts) Trainium2 kernel.

Math (per group g): out_g = x_g @ (sum_e c[g,e] W_e + W_sh).T + (sum_e c[g,e] b_e + b_sh)

Sharding: data-parallel over the 32 groups -> 4 groups (8192 tokens) per core,
expert weights replicated. Host does layout-only prep (transposes / stacking, no
arithmetic); all FLOPs (weight mixing, bias mixing, GEMM, bias add) run on device.

Device plan per core:
  - DMA in: xT [512, 8192] (x shard transposed), WT [9, 512, 512] (transposed
    experts + shared), coefficient broadcast / bias tensors.
  - Mix weights on DVE: WmixT_g = sum_e c[g,e] WT_e + WT_sh via fused
    scalar_tensor_tensor FMAs (8 ops of [128, 2048] per group).
  - Mix biases on PE: tiny K=9 matmul per group.
  - Main GEMM on PE in float32r (1 cyc/row): psum[t128, o512] accumulates 4
    k-tiles plus a K=1 ones-row matmul that adds the mixed bias.
  - Drain PSUM->SBUF on ScalarE, DMA out.
"""
import ml_dtypes
import numpy as np

import concourse.bacc as bacc
import concourse.mybir as mybir
from concourse.alu_op_type import AluOpType
from concourse.tile import TileContext
from concourse.bass_utils import run_bass_kernel_spmd

N_CORES = 8
IN_F = 512
OUT_F = 512
N_EXPERTS = 8
N_GROUPS = 32
TOK_PER_GROUP = 2048
G_PER_CORE = N_GROUPS // N_CORES          # 4
TOK_PER_CORE = G_PER_CORE * TOK_PER_GROUP  # 8192
KT = IN_F // 128                           # 4 k-tiles
F32 = mybir.dt.float32
F32R = mybir.dt.float32r
BF16 = mybir.dt.bfloat16

_CACHE = {}


def _build():
    nc = bacc.Bacc(trn_type="TRN2")
    xT = nc.dram_tensor("xT", (IN_F, TOK_PER_CORE), F32, kind="ExternalInput")
    wt = nc.dram_tensor("wt", (N_EXPERTS + 1, IN_F, OUT_F), F32R, kind="ExternalInput")
    cb = nc.dram_tensor("cb", (128, G_PER_CORE * N_EXPERTS), F32, kind="ExternalInput")
    cx = nc.dram_tensor("cx", (N_EXPERTS + 1, G_PER_CORE), F32R, kind="ExternalInput")
    ball = nc.dram_tensor("ball", (N_EXPERTS + 1, OUT_F), F32R, kind="ExternalInput")
    ones = nc.dram_tensor("ones", (1, 128), BF16, kind="ExternalInput")
    out = nc.dram_tensor("out", (TOK_PER_CORE, OUT_F), F32, kind="ExternalOutput")

    with TileContext(nc) as tc:
        with (
            tc.tile_pool(name="wp", bufs=1) as wp,
            tc.tile_pool(name="mixp", bufs=1) as mixp,
            tc.tile_pool(name="smallp", bufs=1) as smallp,
            tc.tile_pool(name="xp", bufs=3) as xp,
            tc.tile_pool(name="op", bufs=3) as op,
            tc.tile_pool(name="psp", bufs=6, space="PSUM") as psp,
            tc.tile_pool(name="psb", bufs=2, space="PSUM") as psb,
        ):
            # ---- small DMAs first (cheap SP issues, unblock bias/mixing) ----
            cbt = smallp.tile([128, G_PER_CORE * N_EXPERTS], F32, tag="cb")
            nc.sync.dma_start(cbt[:], cb[:])
            cxt = smallp.tile([N_EXPERTS + 1, G_PER_CORE], F32R, tag="cx")
            nc.sync.dma_start(cxt[:], cx[:])
            ballt = smallp.tile([N_EXPERTS + 1, OUT_F], F32R, tag="ball")
            nc.sync.dma_start(ballt[:], ball[:])
            onest = smallp.tile([1, 128], BF16, tag="ones")
            nc.sync.dma_start(onest[:], ones[:])

            # ---- load all 9 experts' k-slice in ONE DMA per k-tile ----
            wt_r = wt[:].rearrange("e (kt p) o -> kt p e o", p=128)  # [4,128,9,512]
            wkt = []
            for kt in range(KT):
                t = wp.tile([128, (N_EXPERTS + 1) * OUT_F], F32R, tag=f"wkt{kt}")
                nc.sync.dma_start(
                    t[:].rearrange("p (e o) -> p e o", e=N_EXPERTS + 1), wt_r[kt]
                )
                wkt.append(t)
            wsb = {
                (e, kt): wkt[kt][:, e * OUT_F : (e + 1) * OUT_F]
                for e in range(N_EXPERTS + 1)
                for kt in range(KT)
            }

            # ---- mixed biases: mb_g = cx[:, g].T @ ball  (K=9, M=1, N=512) ----
            mbt = []
            for g in range(G_PER_CORE):
                pbg = psb.tile([1, OUT_F], F32, tag="pb")
                nc.tensor.matmul(pbg[:], cxt[:, g : g + 1], ballt[:], start=True, stop=True)
                mb = smallp.tile([1, OUT_F], BF16, tag=f"mb{g}")
                nc.vector.tensor_copy(mb[:], pbg[:])
                mbt.append(mb)

            # ---- mix weights on DVE: wmix_g = sum_e c[g,e]*WT_e + WT_sh ----
            # ---- mix per (group, k-tile); final FMA writes bf16 ----
            wmix = {}
            for g in range(G_PER_CORE):
                for kt in range(KT):
                    acc = mixp.tile([128, OUT_F], F32, tag="wma", bufs=2)
                    wm = mixp.tile([128, OUT_F], BF16, tag=f"wm{g}_{kt}")
                    nc.vector.scalar_tensor_tensor(
                        acc[:], wsb[(0, kt)],
                        cbt[:, g * N_EXPERTS : g * N_EXPERTS + 1],
                        wsb[(N_EXPERTS, kt)], AluOpType.mult, AluOpType.add,
                    )
                    for e in range(1, N_EXPERTS):
                        nc.vector.scalar_tensor_tensor(
                            acc[:] if e < N_EXPERTS - 1 else wm[:],
                            wsb[(e, kt)],
                            cbt[:, g * N_EXPERTS + e : g * N_EXPERTS + e + 1],
                            acc[:], AluOpType.mult, AluOpType.add,
                        )
                    wmix[(g, kt)] = wm

            # ---- main GEMM ----
            n_chunks = TOK_PER_CORE // 512  # 16 chunks of 512 tokens
            for ch in range(n_chunks):
                g = ch // (TOK_PER_GROUP // 512)
                t0 = ch * 512
                xs = xp.tile([128, KT * 512], F32, tag="x")
                nc.sync.dma_start(
                    xs[:].rearrange("p (kt t) -> p kt t", kt=KT),
                    xT[:, t0 : t0 + 512].rearrange("(kt p) t -> p kt t", p=128),
                )
                xb = xp.tile([128, KT * 512], BF16, tag="xb")
                nc.scalar.copy(xb[:], xs[:])
                oc = op.tile([128, 4 * OUT_F], F32, tag="o")
                for ts in range(4):
                    ps = psp.tile([128, OUT_F], F32, tag="ps")
                    for kt in range(KT):
                        nc.tensor.matmul(
                            ps[:],
                            xb[:, kt * 512 + ts * 128 : kt * 512 + ts * 128 + 128],
                            wmix[(g, kt)][:],
                            start=(kt == 0),
                            stop=False,
                        )
                    nc.tensor.matmul(ps[:], onest[:], mbt[g][:], start=False, stop=True)
                    nc.scalar.copy(oc[:, ts * OUT_F : (ts + 1) * OUT_F], ps[:])
                nc.sync.dma_start(
                    out[t0 : t0 + 512, :].rearrange("(ts p) o -> p ts o", p=128),
                    oc[:].rearrange("p (ts o) -> p ts o", ts=4),
                )
    nc.finalize()
    return nc


def kernel(x, coefficients, weight_experts, bias_experts, weight_shared, bias_shared, sizes):
    x = np.asarray(x)
    coefficients = np.asarray(coefficients)
    weight_experts = np.asarray(weight_experts)
    bias_experts = np.asarray(bias_experts)
    weight_shared = np.asarray(weight_shared)
    bias_shared = np.asarray(bias_shared)

    if "nc" not in _CACHE:
        _CACHE["nc"] = _build()
    nc = _CACHE["nc"]

    # ---- host-side layout prep (no arithmetic) ----
    wt_np = np.empty((N_EXPERTS + 1, IN_F, OUT_F), np.float32)
    for e in range(N_EXPERTS):
        wt_np[e] = weight_experts[e].T
    wt_np[N_EXPERTS] = weight_shared.T
    ball_np = np.empty((N_EXPERTS + 1, OUT_F), np.float32)
    ball_np[:N_EXPERTS] = bias_experts
    ball_np[N_EXPERTS] = bias_shared
    ones_np = np.ones((1, 128), ml_dtypes.bfloat16)

    in_maps = []
    for c in range(N_CORES):
        gs = slice(c * G_PER_CORE, (c + 1) * G_PER_CORE)
        cg = coefficients[gs]  # [4, 8]
        cb_np = np.broadcast_to(
            cg.reshape(1, -1), (128, G_PER_CORE * N_EXPERTS)
        ).copy()
        cx_np = np.empty((N_EXPERTS + 1, G_PER_CORE), np.float32)
        cx_np[:N_EXPERTS] = cg.T
        cx_np[N_EXPERTS] = 1.0
        xT_np = np.ascontiguousarray(
            x[c * TOK_PER_CORE : (c + 1) * TOK_PER_CORE].T
        )
        in_maps.append(
            {
                "xT": xT_np,
                "wt": wt_np,
                "cb": cb_np,
                "cx": cx_np,
                "ball": ball_np,
                "ones": ones_np,
            }
        )

    res = run_bass_kernel_spmd(nc, in_maps, core_ids=list(range(N_CORES)))
    return np.concatenate([res.results[c]["out"] for c in range(N_CORES)], axis=0)
